# revision 57
# baseline (speedup 1.0000x reference)
"""Trainium2 Bass kernel: GPT2-style windowed attention (DecisionTransformer).

Full-input contract: kernel(**inputs) -> [B, S, D] float32.

Sharding: batch*heads across 8 cores (core c -> batch c//4, heads 4*(c%4)..+4).
Each core: column-sliced c_attn, full windowed attention for its 4 heads,
row-sliced c_proj producing a partial [S, D] output; host sums partials
(the "all-reduce") and adds c_proj bias once.

Per-core device layout choices:
  - hidden is sent pre-transposed (xT [D, S]) so QK projections emit
    qT/kT directly in [head*dim, seq] layout (matmul lhsT = W tiles).
  - V is projected in [seq, head*dim] layout (lhsT = xT tiles) with a
    ones-column appended per head; the attn@V matmul then accumulates
    softmax denominators in PSUM row 64 for free.
  - scores are computed transposed (sT[k, q]) so exp/mask/attn@V never
    need an attention transpose; normalization divides the attn@V output
    by the denominator row (broadcast across partitions via GPSIMD).
  - rope: rotate_half is materialized with 4 small SBUF->SBUF DMAs
    (partition swap), keeping all DVE multiplies full-width.
  - max-subtraction in softmax is skipped: scores are bounded (|s|~3)
    for this problem's scale, so fp32 exp cannot overflow.
"""

import sys

import numpy as np

sys.path.insert(0, "/opt/trn_rl_repo")

B, S, D = 2, 2048, 1024
H, HD = 16, 64
WINDOW = 512
ROPE_BASE = 4000.0
NCORES = 8
NH = 4          # heads per core
KT = D // 128   # 8 contraction tiles for c_attn
NB = S // 128   # 16 seq blocks
WB = WINDOW // 128  # 4 -> band spans up to 5 query blocks per key block


def _build_nc(debug_taps=False):
    import concourse.bass as bass
    from concourse import bacc, library_config, mybir
    import concourse.tile as tile

    f32 = mybir.dt.float32
    f32r = mybir.dt.float32r
    bf16 = mybir.dt.bfloat16
    Exp = mybir.ActivationFunctionType.Exp
    mult = mybir.AluOpType.mult
    ts = bass.ts
    ds = bass.ds

    nc = bacc.Bacc("TRN2")

    xT_d = nc.dram_tensor("xT", [D, S], f32r, kind="ExternalInput")
    wqkv_d = nc.dram_tensor("wqkv", [D, 3 * NH * HD], f32r, kind="ExternalInput")
    bqk_d = nc.dram_tensor("bqk", [128, 4], f32, kind="ExternalInput")
    bv_d = nc.dram_tensor("bv", [1, NH * HD], f32r, kind="ExternalInput")
    wp_d = nc.dram_tensor("wp", [NH * HD, D], f32r, kind="ExternalInput")
    cos2_d = nc.dram_tensor("cos2", [128, S], bf16, kind="ExternalInput")
    sin2_d = nc.dram_tensor("sin2", [128, S], bf16, kind="ExternalInput")
    m0_d = nc.dram_tensor("m0", [128, 128], bf16, kind="ExternalInput")
    m4_d = nc.dram_tensor("m4", [128, 128], bf16, kind="ExternalInput")
    out_d = nc.dram_tensor("out", [S, D], f32, kind="ExternalOutput")
    if debug_taps:
        dbg = {
            "qk0": nc.dram_tensor("dbg_qk0", [128, S + 128], f32, kind="ExternalOutput"),
            "v0": nc.dram_tensor("dbg_v0", [128, NH * 65], bf16, kind="ExternalOutput"),
            "e0": nc.dram_tensor("dbg_e0", [128, 640], bf16, kind="ExternalOutput"),
            "po0": nc.dram_tensor("dbg_po0", [65, S], f32, kind="ExternalOutput"),
            "oh0": nc.dram_tensor("dbg_oh0", [128, S], f32, kind="ExternalOutput"),
        }

    with tile.TileContext(nc) as tc:
        nc.gpsimd.load_library(library_config.attn)

        with (
            tc.tile_pool(name="persist", bufs=1) as pp,
            tc.tile_pool(name="ps", bufs=3, space="PSUM") as ps_pool,
            tc.tile_pool(name="pso", bufs=2, space="PSUM") as pso_pool,
        ):
            bqk_t = pp.tile([128, 4], f32, tag="bqk")
            nc.sync.dma_start(bqk_t[:], bqk_d[:])
            bv_t = pp.tile([1, NH * HD], f32r, tag="bv")
            nc.sync.dma_start(bv_t[:], bv_d[:])
            m0t = pp.tile([128, 128], bf16, tag="m0")
            nc.sync.dma_start(m0t[:], m0_d[:])
            m4t = pp.tile([128, 128], bf16, tag="m4")
            nc.sync.dma_start(m4t[:], m4_d[:])
            # memset can't write f32r; stage in f32 and convert-copy
            z32 = pp.tile([128, 128], f32, tag="z32")
            nc.vector.memset(z32[:], 0.0)
            o32 = pp.tile([1, 128], f32, tag="o32")
            nc.vector.memset(o32[:], 1.0)
            ones1 = pp.tile([1, 128], f32r, tag="ones1")
            nc.vector.tensor_copy(ones1[:], o32[:])

            # qk[c]: c in {0: q heads 01, 1: q heads 23, 2: k heads 01, 3: k heads 23}
            # padded by 128 zero cols so the window-edge scores matmul can run
            # at N=256 (float32r needs N>=256 for full rate)
            SP = S + 128
            qk = [
                pp.tile([128, SP], bf16, tag=f"qk{c}", name=f"qk{c}") for c in range(4)
            ]
            for c in range(4):
                nc.vector.tensor_copy(qk[c][:, S:SP], z32[:])
            CV = NH * 65  # 260: per head 64 v-cols + 1 ones col
            vbig = pp.tile([128, NB, CV], bf16, tag="vbig")
            outH = pp.tile([128, 2, S], f32r, tag="outH")

            # ---- phases B-D scope: x / weights / rope tables (space reclaimed) --
            with (
                tc.tile_pool(name="xw", bufs=1) as xw_pool,
                tc.tile_pool(name="ropetmp", bufs=2) as tmp_pool,
            ):
                # v-columns first: phase D (which runs first) needs only them
                wbig = xw_pool.tile([128, KT, 3 * NH * HD], f32r, tag="wbig")
                VC = 2 * NH * HD
                for kt in range(KT):
                    nc.sync.dma_start(
                        wbig[:, kt, VC:], wqkv_d[ts(kt, 128), VC:]
                    )
                for kt in range(KT):
                    nc.sync.dma_start(
                        wbig[:, kt, 0:VC], wqkv_d[ts(kt, 128), 0:VC]
                    )
                # load x by s-chunks (sc-major) so the first projection psum
                # group is ready after ~2MB instead of the full 8MB
                xbig = xw_pool.tile([128, KT, S], f32r, tag="xbig")
                for sc in range(S // 512):
                    for kt in range(KT):
                        nc.sync.dma_start(
                            xbig[:, kt, ts(sc, 512)], xT_d[ts(kt, 128), ts(sc, 512)]
                        )
                cos2 = xw_pool.tile([128, S], bf16, tag="cos2")
                nc.sync.dma_start(cos2[:], cos2_d[:])
                sin2 = xw_pool.tile([128, S], bf16, tag="sin2")
                nc.sync.dma_start(sin2[:], sin2_d[:])

                # ---- phase D: V projection (out[s, col] = xT^T @ Wv) + ones col --
                for sb in range(NB):
                    vsb = vbig[:, sb, :].rearrange("p (h c) -> p h c", c=65)
                    nc.vector.memset(vsb[:, :, 64:65], 1.0)
                    psv = ps_pool.tile([128, 768], f32, tag="ps", name="psv")
                    for kt in range(KT):
                        nc.tensor.matmul(
                            psv[:, 0 : NH * HD],
                            xbig[:, kt, ts(sb, 128)],
                            wbig[:, kt, ds(2 * NH * HD, NH * HD)],
                            start=(kt == 0),
                            stop=False,
                        )
                    # bias via rank-1 ones x bv accumulate
                    nc.tensor.matmul(
                        psv[:, 0 : NH * HD], ones1[:], bv_t[:],
                        start=False, stop=True,
                    )
                    nc.vector.tensor_copy(
                        vsb[:, :, 0:64],
                        psv[:, 0 : NH * HD].rearrange("p (h c) -> p h c", c=64),
                    )

                # ---- phase B+C: qT/kT projection with rope fused per s-chunk
                # (rope runs on DVE while PE projects the next chunk)
                for sc in range(S // 512):
                    for c in range(4):
                        psb = ps_pool.tile([128, 768], f32, tag="ps", name="psb")
                        for kt in range(KT):
                            nc.tensor.matmul(
                                psb[:, 0:512],
                                wbig[:, kt, ts(c, 128)],
                                xbig[:, kt, ts(sc, 512)],
                                start=(kt == 0),
                                stop=(kt == KT - 1),
                            )
                        # evacuate with per-partition (per-column) bias
                        nc.scalar.add(
                            qk[c][:, ts(sc, 512)], psb[:, 0:512], bqk_t[:, c : c + 1]
                        )
                        # rope this chunk: rotate_half via partition-swap DMAs,
                        # spread across 4 engines' DMA queues (a single queue
                        # serializes 64 partition-sparse copies into a ~26us
                        # tail that stalls PE at the attention boundary)
                        qc = qk[c][:, ts(sc, 512)]
                        tmp = tmp_pool.tile([128, 512], bf16, tag="ropetmp", name="tmp")
                        dma_engs = [nc.sync, nc.gpsimd, nc.scalar, nc.gpsimd]
                        for g in range(2):
                            b0 = g * 64
                            dma_engs[2 * g].dma_start(
                                tmp[b0 : b0 + 32, :], qk[c][b0 + 32 : b0 + 64, ts(sc, 512)]
                            )
                            dma_engs[2 * g + 1].dma_start(
                                tmp[b0 + 32 : b0 + 64, :], qk[c][b0 : b0 + 32, ts(sc, 512)]
                            )
                        # sin-mul on the otherwise-idle GPSIMD so DVE only
                        # does 2 of the 3 rope multiplies (shorter rope tail)
                        nc.gpsimd.tensor_tensor(
                            tmp[:], tmp[:], sin2[:, ts(sc, 512)], op=mult
                        )
                        nc.vector.tensor_tensor(qc, qc, cos2[:, ts(sc, 512)], op=mult)
                        nc.vector.tensor_add(qc, qc, tmp[:])

                if debug_taps:
                    nc.sync.dma_start(dbg["qk0"][:], qk[0][:].bitcast(f32))

            # ---- phases E-F scope ----
            with (
                tc.tile_pool(name="et", bufs=10) as e_pool,
                tc.tile_pool(name="rb", bufs=2) as rb_pool,
                tc.tile_pool(name="yo", bufs=3) as y_pool,
            ):
                # ---- phase E: windowed attention per head ----
                HS = S // 4  # q-quarter span: po is 1 PSUM bank, double-buffered

                def evac_q(po, h, qtr):
                    # normalize by denominators (PSUM row 64) into outH.
                    # custom-DVE must not read PSUM: stage denom row via ACT.
                    hb = (h % 2) * 64
                    rb = rb_pool.tile([64, HS], f32, tag="rb", name="rb")
                    nc.scalar.copy(rb[0:1, :], po[64:65, :])
                    nc.vector.reciprocal_approx_fast(rb[0:1, :], rb[0:1, :])
                    nc.gpsimd.partition_broadcast(rb[:], rb[0:1, :])
                    nc.vector.tensor_tensor(
                        outH[hb : hb + 64, h // 2, qtr * HS : (qtr + 1) * HS],
                        po[0:64, :],
                        rb[:],
                        op=mult,
                    )

                for h in range(NH):
                    hb = (h % 2) * 64
                    qt = qk[h // 2]
                    kt_ = qk[2 + h // 2]
                    eTs = {}  # kj -> exp'd/masked transposed scores [128, 640]

                    def scores_exp(kj, h=h):
                        nq = min(WB + 1, NB - kj)
                        pss = ps_pool.tile([128, 768], f32, tag="ps", name="pss")
                        n1 = min(512, nq * 128)
                        n2 = nq * 128 - n1
                        lhs_k = kt_[hb : hb + 64, ts(kj, 128)]
                        nc.tensor.matmul(
                            pss[:, 0:n1],
                            lhs_k,
                            qt[hb : hb + 64, ds(kj * 128, n1)],
                            start=True,
                            stop=True,
                        )
                        if n2:
                            nc.tensor.matmul(
                                pss[:, 512 : 512 + n2],
                                lhs_k,
                                qt[hb : hb + 64, ds(kj * 128 + 512, n2)],
                                start=True,
                                stop=True,
                            )
                        eT = e_pool.tile([128, 640], bf16, tag="et", name="eT")
                        nc.scalar.activation(
                            eT[:, 0 : nq * 128], pss[:, 0 : nq * 128], Exp, scale=0.125
                        )
                        # banded mask: diag block keeps kk<=qq, window edge kk>qq
                        nc.vector.tensor_tensor(
                            eT[:, 0:128], eT[:, 0:128], m0t[:], op=mult
                        )
                        if nq == WB + 1:
                            nc.vector.tensor_tensor(
                                eT[:, 512:640], eT[:, 512:640], m4t[:], op=mult
                            )
                        eTs[kj] = eT

                    # qi-major so each po block's psum accumulation group opens
                    # and closes within one iteration (bank zero-region rule);
                    # scores run 3 iterations ahead of attn@V (pss bufs=3) so
                    # PE never stalls on ACT's exp latency. po covers one
                    # q-quarter (1 bank, bufs=2) so normalization of a
                    # finished quarter overlaps attn@V of the next.
                    scores_exp(0)
                    scores_exp(1)
                    scores_exp(2)
                    po = None
                    QB = NB // 4  # 4 q-blocks per quarter
                    for qi in range(NB):
                        if qi + 3 < NB:
                            scores_exp(qi + 3)
                        if qi % QB == 0:
                            if po is not None:
                                evac_q(po, h, qi // QB - 1)
                            po = pso_pool.tile([65, HS], f32, tag="pso", name="po")
                        kjlo = max(0, qi - WB)
                        for kj in range(kjlo, qi + 1):
                            nc.tensor.matmul(
                                po[:, ts(qi % QB, 128)],
                                vbig[:, kj, h * 65 : h * 65 + 65],
                                eTs[kj][:, ts(qi - kj, 128)],
                                start=(kj == kjlo),
                                stop=(kj == qi),
                            )
                    evac_q(po, h, 3)

                    if debug_taps and h == 0:
                        nc.sync.dma_start(dbg["v0"][:], vbig[:, 0, :])
                        nc.sync.dma_start(dbg["e0"][:], eTs[0][:])

                if debug_taps:
                    nc.sync.dma_start(dbg["oh0"][:], outH[:, 0, :].bitcast(f32))

                # ---- phase F: c_proj (row-parallel slice) ----
                wpt = pp.tile([128, 2, D], f32r, tag="wpt")
                for k2 in range(2):
                    nc.sync.dma_start(wpt[:, k2, :], wp_d[ts(k2, 128), :])
                for sb in range(NB):
                    psp = ps_pool.tile([128, 1024], f32, tag="ps", name="psp")
                    for k2 in range(2):
                        for ncol in range(2):
                            nc.tensor.matmul(
                                psp[:, ts(ncol, 512)],
                                outH[:, k2, ts(sb, 128)],
                                wpt[:, k2, ts(ncol, 512)],
                                start=(k2 == 0),
                                stop=(k2 == 1),
                            )
                    yt = y_pool.tile([128, D], f32, tag="yo", name="yt")
                    nc.scalar.copy(yt[:], psp[:])
                    nc.sync.dma_start(out_d[ts(sb, 128), :], yt[:])

    nc.compile()
    return nc


def _host_inputs(hidden, pos, caw, cab, cpw):
    """Build the 8 per-core input maps."""
    inv = 1.0 / (ROPE_BASE ** (np.arange(0, HD, 2, dtype=np.float32) / HD))
    t = np.arange(S, dtype=np.float32)
    freqs = np.outer(t, inv).astype(np.float32)
    emb = np.concatenate([freqs, freqs], axis=1)  # [S, HD]
    cos = np.cos(emb).astype(np.float32)
    sin = np.sin(emb).astype(np.float32)

    import ml_dtypes

    ii = np.arange(128)
    m0 = (ii[:, None] <= ii[None, :]).astype(ml_dtypes.bfloat16)
    m4 = (ii[:, None] > ii[None, :]).astype(ml_dtypes.bfloat16)

    xTs, cos2s, sin2s = [], [], []
    for b in range(B):
        xTs.append(np.ascontiguousarray(hidden[b].T))
        cosT = np.ascontiguousarray(cos[pos[b]].T)  # [HD, S]
        sinT = np.ascontiguousarray(sin[pos[b]].T)
        sinS = np.concatenate([-sinT[:32], sinT[32:]], axis=0)
        cos2s.append(np.tile(cosT, (2, 1)).astype(ml_dtypes.bfloat16))
        sin2s.append(np.tile(sinS, (2, 1)).astype(ml_dtypes.bfloat16))

    in_maps = []
    for c in range(NCORES):
        b = c // 4
        h0 = NH * (c % 4)
        col = h0 * HD
        w_q = caw[:, col : col + NH * HD]
        w_k = caw[:, D + col : D + col + NH * HD]
        w_v = caw[:, 2 * D + col : 2 * D + col + NH * HD]
        wqkv = np.ascontiguousarray(np.concatenate([w_q, w_k, w_v], axis=1))
        b_q = cab[col : col + NH * HD]
        b_k = cab[D + col : D + col + NH * HD]
        bqk = np.ascontiguousarray(
            np.concatenate([b_q, b_k]).reshape(4, 128).T
        )  # [128, 4]: partition = col within tile
        bv = np.ascontiguousarray(
            cab[2 * D + col : 2 * D + col + NH * HD].reshape(1, -1)
        )
        wp = np.ascontiguousarray(cpw[col : col + NH * HD, :])
        in_maps.append(
            {
                "xT": xTs[b],
                "wqkv": wqkv,
                "bqk": bqk,
                "bv": bv,
                "wp": wp,
                "cos2": cos2s[b],
                "sin2": sin2s[b],
                "m0": m0,
                "m4": m4,
            }
        )
    return in_maps


def _assemble(results, cpb):
    """Host all-reduce of the 4 per-batch partials + c_proj bias."""
    y = np.empty((B, S, D), dtype=np.float32)
    for b in range(B):
        acc = results[4 * b]["out"].astype(np.float32)
        for c in range(4 * b + 1, 4 * b + 4):
            acc = acc + results[c]["out"]
        y[b] = acc + cpb[None, :]
    return y


def kernel(**inputs):
    from concourse import bass_utils

    hidden = np.asarray(inputs["hidden_states"], dtype=np.float32)
    pos = np.asarray(inputs["position_ids"]).astype(np.int64)
    caw = np.asarray(inputs["c_attn_w"], dtype=np.float32)
    cab = np.asarray(inputs["c_attn_b"], dtype=np.float32)
    cpw = np.asarray(inputs["c_proj_w"], dtype=np.float32)
    cpb = np.asarray(inputs["c_proj_b"], dtype=np.float32)

    in_maps = _host_inputs(hidden, pos, caw, cab, cpw)
    nc = _build_nc()
    res = bass_utils.run_bass_kernel_spmd(nc, in_maps, list(range(NCORES)))
    return _assemble(res.results, cpb)


# revision 58
# speedup vs baseline: 1.1454x; 1.1454x over previous
"""Trainium2 Bass kernel: GPT2-style windowed attention (DecisionTransformer).

Full-input contract: kernel(**inputs) -> [B, S, D] float32.

Sharding: batch*heads across 8 cores (core c -> batch c//4, heads 4*(c%4)..+4).
Each core: column-sliced c_attn, full windowed attention for its 4 heads,
row-sliced c_proj producing a partial [S, D] output; host sums partials
(the "all-reduce") and adds c_proj bias once.

Per-core device layout choices:
  - hidden is sent pre-transposed (xT [D, S]) so QK projections emit
    qT/kT directly in [head*dim, seq] layout (matmul lhsT = W tiles).
  - V is projected in [seq, head*dim] layout (lhsT = xT tiles) with a
    ones-column appended per head; the attn@V matmul then accumulates
    softmax denominators in PSUM row 64 for free.
  - scores are computed transposed (sT[k, q]) so exp/mask/attn@V never
    need an attention transpose; normalization divides the attn@V output
    by the denominator row (broadcast across partitions via GPSIMD).
  - rope: rotate_half is materialized with 4 small SBUF->SBUF DMAs
    (partition swap), keeping all DVE multiplies full-width.
  - max-subtraction in softmax is skipped: scores are bounded (|s|~3)
    for this problem's scale, so fp32 exp cannot overflow.
"""

import sys

import numpy as np

sys.path.insert(0, "/opt/trn_rl_repo")

B, S, D = 2, 2048, 1024
H, HD = 16, 64
WINDOW = 512
ROPE_BASE = 4000.0
NCORES = 8
NH = 4          # heads per core
KT = D // 128   # 8 contraction tiles for c_attn
NB = S // 128   # 16 seq blocks
WB = WINDOW // 128  # 4 -> band spans up to 5 query blocks per key block


def _build_nc(debug_taps=False):
    import concourse.bass as bass
    from concourse import bacc, library_config, mybir
    import concourse.tile as tile

    f32 = mybir.dt.float32
    f32r = mybir.dt.float32r
    bf16 = mybir.dt.bfloat16
    Exp = mybir.ActivationFunctionType.Exp
    mult = mybir.AluOpType.mult
    ts = bass.ts
    ds = bass.ds

    nc = bacc.Bacc("TRN2")

    xT_d = nc.dram_tensor("xT", [D, S], f32r, kind="ExternalInput")
    wqkv_d = nc.dram_tensor("wqkv", [D, 3 * NH * HD], f32r, kind="ExternalInput")
    bqk_d = nc.dram_tensor("bqk", [128, 4], f32, kind="ExternalInput")
    bv_d = nc.dram_tensor("bv", [1, NH * HD], f32r, kind="ExternalInput")
    wp_d = nc.dram_tensor("wp", [NH * HD, D], f32r, kind="ExternalInput")
    cos2_d = nc.dram_tensor("cos2", [128, S], bf16, kind="ExternalInput")
    sin2_d = nc.dram_tensor("sin2", [128, S], bf16, kind="ExternalInput")
    m0_d = nc.dram_tensor("m0", [128, 128], bf16, kind="ExternalInput")
    m4_d = nc.dram_tensor("m4", [128, 128], bf16, kind="ExternalInput")
    out_d = nc.dram_tensor("out", [S, D], f32, kind="ExternalOutput")
    if debug_taps:
        dbg = {
            "qk0": nc.dram_tensor("dbg_qk0", [128, S + 128], f32, kind="ExternalOutput"),
            "v0": nc.dram_tensor("dbg_v0", [128, NH * 65], bf16, kind="ExternalOutput"),
            "e0": nc.dram_tensor("dbg_e0", [128, 640], bf16, kind="ExternalOutput"),
            "po0": nc.dram_tensor("dbg_po0", [65, S], f32, kind="ExternalOutput"),
            "oh0": nc.dram_tensor("dbg_oh0", [128, S], f32, kind="ExternalOutput"),
        }

    with tile.TileContext(nc) as tc:
        nc.gpsimd.load_library(library_config.attn)

        with (
            tc.tile_pool(name="persist", bufs=1) as pp,
            tc.tile_pool(name="ps", bufs=3, space="PSUM") as ps_pool,
            tc.tile_pool(name="pso", bufs=2, space="PSUM") as pso_pool,
        ):
            bqk_t = pp.tile([128, 4], f32, tag="bqk")
            nc.sync.dma_start(bqk_t[:], bqk_d[:])
            bv_t = pp.tile([1, NH * HD], f32r, tag="bv")
            nc.sync.dma_start(bv_t[:], bv_d[:])
            m0t = pp.tile([128, 128], bf16, tag="m0")
            nc.sync.dma_start(m0t[:], m0_d[:])
            m4t = pp.tile([128, 128], bf16, tag="m4")
            nc.sync.dma_start(m4t[:], m4_d[:])
            # memset can't write f32r; stage in f32 and convert-copy
            z32 = pp.tile([128, 128], f32, tag="z32")
            nc.vector.memset(z32[:], 0.0)
            o32 = pp.tile([1, 128], f32, tag="o32")
            nc.vector.memset(o32[:], 1.0)
            ones1 = pp.tile([1, 128], f32r, tag="ones1")
            nc.vector.tensor_copy(ones1[:], o32[:])

            # qk[c]: c in {0: q heads 01, 1: q heads 23, 2: k heads 01, 3: k heads 23}
            # padded by 128 zero cols so the window-edge scores matmul can run
            # at N=256 (float32r needs N>=256 for full rate)
            SP = S + 128
            qk = [
                pp.tile([128, SP], bf16, tag=f"qk{c}", name=f"qk{c}") for c in range(4)
            ]
            for c in range(4):
                nc.vector.tensor_copy(qk[c][:, S:SP], z32[:])
            CV = NH * 65  # 260: per head 64 v-cols + 1 ones col
            vbig = pp.tile([128, NB, CV], bf16, tag="vbig")
            outH = pp.tile([128, 2, S], f32r, tag="outH")

            # ---- phases B-D scope: x / weights / rope tables (space reclaimed) --
            with (
                tc.tile_pool(name="xw", bufs=1) as xw_pool,
                tc.tile_pool(name="ropetmp", bufs=2) as tmp_pool,
            ):
                # v-columns first: phase D (which runs first) needs only them
                wbig = xw_pool.tile([128, KT, 3 * NH * HD], f32r, tag="wbig")
                VC = 2 * NH * HD
                for kt in range(KT):
                    nc.sync.dma_start(
                        wbig[:, kt, VC:], wqkv_d[ts(kt, 128), VC:]
                    )
                for kt in range(KT):
                    nc.sync.dma_start(
                        wbig[:, kt, 0:VC], wqkv_d[ts(kt, 128), 0:VC]
                    )
                # load x by s-chunks (sc-major) so the first projection psum
                # group is ready after ~2MB instead of the full 8MB
                xbig = xw_pool.tile([128, KT, S], f32r, tag="xbig")
                for sc in range(S // 512):
                    for kt in range(KT):
                        nc.sync.dma_start(
                            xbig[:, kt, ts(sc, 512)], xT_d[ts(kt, 128), ts(sc, 512)]
                        )
                cos2 = xw_pool.tile([128, S], bf16, tag="cos2")
                nc.sync.dma_start(cos2[:], cos2_d[:])
                sin2 = xw_pool.tile([128, S], bf16, tag="sin2")
                nc.sync.dma_start(sin2[:], sin2_d[:])

                # ---- phase D: V projection (out[s, col] = xT^T @ Wv) + ones col --
                for sb in range(NB):
                    vsb = vbig[:, sb, :].rearrange("p (h c) -> p h c", c=65)
                    nc.vector.memset(vsb[:, :, 64:65], 1.0)
                    psv = ps_pool.tile([128, 768], f32, tag="ps", name="psv")
                    for kt in range(KT):
                        nc.tensor.matmul(
                            psv[:, 0 : NH * HD],
                            xbig[:, kt, ts(sb, 128)],
                            wbig[:, kt, ds(2 * NH * HD, NH * HD)],
                            start=(kt == 0),
                            stop=False,
                        )
                    # bias via rank-1 ones x bv accumulate
                    nc.tensor.matmul(
                        psv[:, 0 : NH * HD], ones1[:], bv_t[:],
                        start=False, stop=True,
                    )
                    nc.vector.tensor_copy(
                        vsb[:, :, 0:64],
                        psv[:, 0 : NH * HD].rearrange("p (h c) -> p h c", c=64),
                    )

                # ---- phase B+C: qT/kT projection with rope fused per s-chunk
                # (rope runs on DVE while PE projects the next chunk)
                for sc in range(S // 512):
                    for c in range(4):
                        psb = ps_pool.tile([128, 768], f32, tag="ps", name="psb")
                        for kt in range(KT):
                            nc.tensor.matmul(
                                psb[:, 0:512],
                                wbig[:, kt, ts(c, 128)],
                                xbig[:, kt, ts(sc, 512)],
                                start=(kt == 0),
                                stop=(kt == KT - 1),
                            )
                        # evacuate with per-partition (per-column) bias
                        nc.scalar.add(
                            qk[c][:, ts(sc, 512)], psb[:, 0:512], bqk_t[:, c : c + 1]
                        )
                        # rope this chunk: rotate_half via partition-swap DMAs,
                        # spread across 4 engines' DMA queues (a single queue
                        # serializes 64 partition-sparse copies into a ~26us
                        # tail that stalls PE at the attention boundary)
                        qc = qk[c][:, ts(sc, 512)]
                        tmp = tmp_pool.tile([128, 512], bf16, tag="ropetmp", name="tmp")
                        dma_engs = [nc.sync, nc.gpsimd, nc.scalar, nc.gpsimd]
                        for g in range(2):
                            b0 = g * 64
                            dma_engs[2 * g].dma_start(
                                tmp[b0 : b0 + 32, :], qk[c][b0 + 32 : b0 + 64, ts(sc, 512)]
                            )
                            dma_engs[2 * g + 1].dma_start(
                                tmp[b0 + 32 : b0 + 64, :], qk[c][b0 : b0 + 32, ts(sc, 512)]
                            )
                        nc.vector.tensor_tensor(
                            tmp[:], tmp[:], sin2[:, ts(sc, 512)], op=mult
                        )
                        nc.vector.tensor_tensor(qc, qc, cos2[:, ts(sc, 512)], op=mult)
                        nc.vector.tensor_add(qc, qc, tmp[:])

                if debug_taps:
                    nc.sync.dma_start(dbg["qk0"][:], qk[0][:].bitcast(f32))

            # ---- phases E-F scope ----
            with (
                tc.tile_pool(name="et", bufs=10) as e_pool,
                tc.tile_pool(name="rb", bufs=2) as rb_pool,
                tc.tile_pool(name="yo", bufs=3) as y_pool,
            ):
                # ---- phase E: windowed attention per head ----
                HS = S // 4  # q-quarter span: po is 1 PSUM bank, double-buffered

                def evac_q(po, h, qtr):
                    # normalize by denominators (PSUM row 64) into outH.
                    # custom-DVE must not read PSUM: stage denom row via ACT.
                    hb = (h % 2) * 64
                    rb = rb_pool.tile([64, HS], f32, tag="rb", name="rb")
                    nc.scalar.copy(rb[0:1, :], po[64:65, :])
                    nc.vector.reciprocal_approx_fast(rb[0:1, :], rb[0:1, :])
                    nc.gpsimd.partition_broadcast(rb[:], rb[0:1, :])
                    nc.vector.tensor_tensor(
                        outH[hb : hb + 64, h // 2, qtr * HS : (qtr + 1) * HS],
                        po[0:64, :],
                        rb[:],
                        op=mult,
                    )

                for h in range(NH):
                    hb = (h % 2) * 64
                    qt = qk[h // 2]
                    kt_ = qk[2 + h // 2]
                    eTs = {}  # kj -> exp'd/masked transposed scores [128, 640]

                    def scores_exp(kj, h=h):
                        nq = min(WB + 1, NB - kj)
                        pss = ps_pool.tile([128, 768], f32, tag="ps", name="pss")
                        n1 = min(512, nq * 128)
                        n2 = nq * 128 - n1
                        lhs_k = kt_[hb : hb + 64, ts(kj, 128)]
                        nc.tensor.matmul(
                            pss[:, 0:n1],
                            lhs_k,
                            qt[hb : hb + 64, ds(kj * 128, n1)],
                            start=True,
                            stop=True,
                        )
                        if n2:
                            nc.tensor.matmul(
                                pss[:, 512 : 512 + n2],
                                lhs_k,
                                qt[hb : hb + 64, ds(kj * 128 + 512, n2)],
                                start=True,
                                stop=True,
                            )
                        eT = e_pool.tile([128, 640], bf16, tag="et", name="eT")
                        nc.scalar.activation(
                            eT[:, 0 : nq * 128], pss[:, 0 : nq * 128], Exp, scale=0.125
                        )
                        # banded mask: diag block keeps kk<=qq, window edge kk>qq
                        nc.vector.tensor_tensor(
                            eT[:, 0:128], eT[:, 0:128], m0t[:], op=mult
                        )
                        if nq == WB + 1:
                            nc.vector.tensor_tensor(
                                eT[:, 512:640], eT[:, 512:640], m4t[:], op=mult
                            )
                        eTs[kj] = eT

                    # qi-major so each po block's psum accumulation group opens
                    # and closes within one iteration (bank zero-region rule);
                    # scores run 3 iterations ahead of attn@V (pss bufs=3) so
                    # PE never stalls on ACT's exp latency. po covers one
                    # q-quarter (1 bank, bufs=2) so normalization of a
                    # finished quarter overlaps attn@V of the next.
                    scores_exp(0)
                    scores_exp(1)
                    scores_exp(2)
                    po = None
                    QB = NB // 4  # 4 q-blocks per quarter
                    for qi in range(NB):
                        if qi + 3 < NB:
                            scores_exp(qi + 3)
                        if qi % QB == 0:
                            if po is not None:
                                evac_q(po, h, qi // QB - 1)
                            po = pso_pool.tile([65, HS], f32, tag="pso", name="po")
                        kjlo = max(0, qi - WB)
                        for kj in range(kjlo, qi + 1):
                            nc.tensor.matmul(
                                po[:, ts(qi % QB, 128)],
                                vbig[:, kj, h * 65 : h * 65 + 65],
                                eTs[kj][:, ts(qi - kj, 128)],
                                start=(kj == kjlo),
                                stop=(kj == qi),
                            )
                    evac_q(po, h, 3)

                    if debug_taps and h == 0:
                        nc.sync.dma_start(dbg["v0"][:], vbig[:, 0, :])
                        nc.sync.dma_start(dbg["e0"][:], eTs[0][:])

                if debug_taps:
                    nc.sync.dma_start(dbg["oh0"][:], outH[:, 0, :].bitcast(f32))

                # ---- phase F: c_proj (row-parallel slice) ----
                wpt = pp.tile([128, 2, D], f32r, tag="wpt")
                for k2 in range(2):
                    nc.sync.dma_start(wpt[:, k2, :], wp_d[ts(k2, 128), :])
                for sb in range(NB):
                    psp = ps_pool.tile([128, 1024], f32, tag="ps", name="psp")
                    for k2 in range(2):
                        for ncol in range(2):
                            nc.tensor.matmul(
                                psp[:, ts(ncol, 512)],
                                outH[:, k2, ts(sb, 128)],
                                wpt[:, k2, ts(ncol, 512)],
                                start=(k2 == 0),
                                stop=(k2 == 1),
                            )
                    yt = y_pool.tile([128, D], f32, tag="yo", name="yt")
                    nc.scalar.copy(yt[:], psp[:])
                    nc.sync.dma_start(out_d[ts(sb, 128), :], yt[:])

    nc.compile()
    return nc


def _host_inputs(hidden, pos, caw, cab, cpw):
    """Build the 8 per-core input maps."""
    inv = 1.0 / (ROPE_BASE ** (np.arange(0, HD, 2, dtype=np.float32) / HD))
    t = np.arange(S, dtype=np.float32)
    freqs = np.outer(t, inv).astype(np.float32)
    emb = np.concatenate([freqs, freqs], axis=1)  # [S, HD]
    cos = np.cos(emb).astype(np.float32)
    sin = np.sin(emb).astype(np.float32)

    import ml_dtypes

    ii = np.arange(128)
    m0 = (ii[:, None] <= ii[None, :]).astype(ml_dtypes.bfloat16)
    m4 = (ii[:, None] > ii[None, :]).astype(ml_dtypes.bfloat16)

    xTs, cos2s, sin2s = [], [], []
    for b in range(B):
        xTs.append(np.ascontiguousarray(hidden[b].T))
        cosT = np.ascontiguousarray(cos[pos[b]].T)  # [HD, S]
        sinT = np.ascontiguousarray(sin[pos[b]].T)
        sinS = np.concatenate([-sinT[:32], sinT[32:]], axis=0)
        cos2s.append(np.tile(cosT, (2, 1)).astype(ml_dtypes.bfloat16))
        sin2s.append(np.tile(sinS, (2, 1)).astype(ml_dtypes.bfloat16))

    in_maps = []
    for c in range(NCORES):
        b = c // 4
        h0 = NH * (c % 4)
        col = h0 * HD
        w_q = caw[:, col : col + NH * HD]
        w_k = caw[:, D + col : D + col + NH * HD]
        w_v = caw[:, 2 * D + col : 2 * D + col + NH * HD]
        wqkv = np.ascontiguousarray(np.concatenate([w_q, w_k, w_v], axis=1))
        b_q = cab[col : col + NH * HD]
        b_k = cab[D + col : D + col + NH * HD]
        bqk = np.ascontiguousarray(
            np.concatenate([b_q, b_k]).reshape(4, 128).T
        )  # [128, 4]: partition = col within tile
        bv = np.ascontiguousarray(
            cab[2 * D + col : 2 * D + col + NH * HD].reshape(1, -1)
        )
        wp = np.ascontiguousarray(cpw[col : col + NH * HD, :])
        in_maps.append(
            {
                "xT": xTs[b],
                "wqkv": wqkv,
                "bqk": bqk,
                "bv": bv,
                "wp": wp,
                "cos2": cos2s[b],
                "sin2": sin2s[b],
                "m0": m0,
                "m4": m4,
            }
        )
    return in_maps


def _assemble(results, cpb):
    """Host all-reduce of the 4 per-batch partials + c_proj bias."""
    y = np.empty((B, S, D), dtype=np.float32)
    for b in range(B):
        acc = results[4 * b]["out"].astype(np.float32)
        for c in range(4 * b + 1, 4 * b + 4):
            acc = acc + results[c]["out"]
        y[b] = acc + cpb[None, :]
    return y


def kernel(**inputs):
    from concourse import bass_utils

    hidden = np.asarray(inputs["hidden_states"], dtype=np.float32)
    pos = np.asarray(inputs["position_ids"]).astype(np.int64)
    caw = np.asarray(inputs["c_attn_w"], dtype=np.float32)
    cab = np.asarray(inputs["c_attn_b"], dtype=np.float32)
    cpw = np.asarray(inputs["c_proj_w"], dtype=np.float32)
    cpb = np.asarray(inputs["c_proj_b"], dtype=np.float32)

    in_maps = _host_inputs(hidden, pos, caw, cab, cpw)
    nc = _build_nc()
    res = bass_utils.run_bass_kernel_spmd(nc, in_maps, list(range(NCORES)))
    return _assemble(res.results, cpb)
